# revision 1
# baseline (speedup 1.0000x reference)
"""CTC loss kernel for Trainium2, 8-core data-parallel.

Contract: kernel(**inputs) takes FULL inputs (log_probs [256,64,8000] f32,
targets [64,32] int, input_lengths [64] int, target_lengths [64] int) and
returns the scalar mean loss (f32), matching reference.reference().

Strategy (per core, 8 samples):
  - DMA the log_probs shard in 16 chunks of [16 t x 8 b x 8000 c], laid out
    with partitions = (b, tau) so each 16-partition group holds one sample.
  - GPSIMD ap_gather extracts the 65 needed classes (blank-interleaved
    targets) per sample -> em [(b,tau), 80].
  - Per-partition max (negated) + ACT exp(em - m - 1e9*dead) turns log-probs
    into normalized probabilities; dead timesteps (t >= input_length) become
    a one-hot at the final blank state so the recursion preserves the answer
    (no per-sample end capture needed).
  - Bounce through a DRAM scratch to transpose partitions (b,tau) -> b,
    giving em_all [8 part(b), 256 t, 80 s] in SBUF.
  - Sequential CTC forward recursion in linear (probability) domain:
    4 DVE ops per timestep on [8, 65] tiles, rescaling by 1/sum every 16
    steps (validated: alpha max stays >= ~1e-19 between rescales).
  - Final: masked sum -> per-sample -(log v + sum log scales); the per-t max
    log corrections are shipped to the host as per-lane partial sums.
Host: shard/unshard, tiny index/mask table construction, final
loss = part1 + sum(Cacc), mean(loss / max(target_lengths,1)).
"""

from contextlib import ExitStack

import numpy as np

T_MAX, B_FULL, C_CLS = 256, 64, 8000
L_MAX = 32
S = 2 * L_MAX + 1            # 65 states
N_CORES = 8
B_LOC = B_FULL // N_CORES    # 8 samples per core
TCHUNK = 16                  # timesteps per DMA chunk
NCHUNK = T_MAX // TCHUNK     # 16
NIDX = 80                    # gather idxs per sample (65 used, mult of 16)
AW = S + 7                   # alpha tile free width (72): states at 2..66
RESCALE_K = 16

_BUILD_CACHE = {}


def _build_program(variant="full", repeat=1):
    import concourse.bacc as bacc
    import concourse.tile as tile
    from concourse import mybir

    f32 = mybir.dt.float32

    nc = bacc.Bacc("TRN2", target_bir_lowering=False, debug=False)

    lp = nc.dram_tensor("lp", [T_MAX, B_LOC, C_CLS], f32, kind="ExternalInput").ap()
    idxtab = nc.dram_tensor("idxtab", [128, NIDX // 16], mybir.dt.int16,
                            kind="ExternalInput").ap()
    bias2 = nc.dram_tensor("bias2", [128, NCHUNK], f32, kind="ExternalInput").ap()
    mlive = nc.dram_tensor("mlive", [128, NCHUNK], f32, kind="ExternalInput").ap()
    deadhot = nc.dram_tensor("deadhot", [128, NCHUNK, NIDX], f32,
                             kind="ExternalInput").ap()
    kkpad = nc.dram_tensor("kkpad", [B_LOC, AW], f32, kind="ExternalInput").ap()
    wmask = nc.dram_tensor("wmask", [B_LOC, AW], f32, kind="ExternalInput").ap()
    out_part = nc.dram_tensor("loss_part", [B_LOC], f32, kind="ExternalOutput").ap()
    out_cacc = nc.dram_tensor("cacc", [128], f32, kind="ExternalOutput").ap()

    n_resc = (T_MAX - 1) // RESCALE_K   # rescale after t = 16,32,...,240

    with tile.TileContext(nc) as tc, ExitStack() as ctx:
        lpp = ctx.enter_context(tc.tile_pool(name="lpp", bufs=3))
        gp = ctx.enter_context(tc.tile_pool(name="gp", bufs=3))
        smalls = ctx.enter_context(tc.tile_pool(name="smalls", bufs=1))
        chsm = ctx.enter_context(tc.tile_pool(name="chsm", bufs=3))
        emp = ctx.enter_context(tc.tile_pool(name="emp", bufs=1))

        # --- one-time small loads ---
        t_idx = smalls.tile([128, NIDX // 16], mybir.dt.int16)
        t_bias2 = smalls.tile([128, NCHUNK], f32)
        t_mlive = smalls.tile([128, NCHUNK], f32)
        t_deadhot = smalls.tile([128, NCHUNK, NIDX], f32)
        t_kk = smalls.tile([B_LOC, AW], f32)
        t_wm = smalls.tile([B_LOC, AW], f32)
        nc.sync.dma_start(out=t_idx, in_=idxtab)
        nc.sync.dma_start(out=t_bias2, in_=bias2)
        nc.sync.dma_start(out=t_mlive, in_=mlive)
        nc.sync.dma_start(out=t_deadhot, in_=deadhot)
        nc.sync.dma_start(out=t_kk, in_=kkpad)
        nc.sync.dma_start(out=t_wm, in_=wmask)

        # --- persistent state ---
        em_all = emp.tile([B_LOC, T_MAX, NIDX], f32)      # 80KB/partition
        t_cacc = smalls.tile([128, 1], f32)
        t_A = smalls.tile([B_LOC, AW], f32)
        t_A2 = smalls.tile([B_LOC, AW], f32)
        t_u = smalls.tile([B_LOC, S], f32)
        t_w = smalls.tile([B_LOC, S], f32)
        t_shist = smalls.tile([B_LOC, n_resc + 1], f32)
        t_r = smalls.tile([B_LOC, 1], f32)
        # Pool-engine wait absorber for the idx-table DMA: the APGather ISA
        # struct cannot carry sem waits, so a regular Pool instruction must
        # observe every dependency first. (Library loads for ap_gather are
        # auto-inserted by Bacc.compile().)
        t_touch = smalls.tile([128, 1], mybir.dt.int16)
        nc.gpsimd.tensor_copy(out=t_touch, in_=t_idx[:, 0:1])

        def prep_chunk(c):
            d = lpp.tile([128, C_CLS], f32, tag="lp")
            src = lp[c * TCHUNK:(c + 1) * TCHUNK, :, :].rearrange(
                "t b c -> b t c")
            nc.sync.dma_start(out=d, in_=src)
            g = gp.tile([128, NIDX], f32, tag="g")
            if variant in ("nogather", "prep_only"):
                nc.vector.tensor_copy(out=g, in_=d[:, 0:NIDX])
            elif variant == "fastgather":
                # timing probe only: 36-idx gather (wrong results)
                nc.gpsimd.tensor_copy(out=g[:, 0:1], in_=d[:, 0:1])
                nc.gpsimd.ap_gather(out_ap=g[:, 0:36], in_ap=d, idxs_ap=t_idx,
                                    channels=128, num_elems=C_CLS, d=1,
                                    num_idxs=36)
            else:
                # wait-carrier: absorbs the lp-DMA wait and g's WAR waits so
                # the APGather ISA instruction itself needs no sem waits.
                nc.gpsimd.tensor_copy(out=g[:, 0:1], in_=d[:, 0:1])
                nc.gpsimd.ap_gather(out_ap=g, in_ap=d, idxs_ap=t_idx,
                                    channels=128, num_elems=C_CLS, d=1,
                                    num_idxs=NIDX)
            mneg = chsm.tile([128, 1], f32, tag="mneg")
            nc.vector.tensor_reduce(out=mneg, in_=g, axis=mybir.AxisListType.X,
                                    op=mybir.AluOpType.max, negate=True)
            bd = chsm.tile([128, 1], f32, tag="bd")
            nc.vector.tensor_add(out=bd, in0=mneg, in1=t_bias2[:, c:c + 1])
            p = gp.tile([128, NIDX], f32, tag="p")
            nc.scalar.activation(out=p, in_=g,
                                 func=mybir.ActivationFunctionType.Exp,
                                 bias=bd, scale=1.0)
            nc.vector.tensor_add(out=p, in0=p, in1=t_deadhot[:, c, :])
            # cacc += mneg * mask_live (excludes dead t from the correction)
            nc.vector.scalar_tensor_tensor(out=t_cacc, in0=mneg,
                                           scalar=t_mlive[:, c:c + 1],
                                           in1=t_cacc,
                                           op0=mybir.AluOpType.mult,
                                           op1=mybir.AluOpType.add)
            # partition reshuffle [(b,tau), 80] -> [b, tau, s] via SBUF->SBUF
            # DMA on the ACT HWDGE ring (keeps the SP ring a pure stream of
            # big lp loads -- HWDGE rings block in-order on unmet waits).
            nc.scalar.dma_start(
                out=em_all[:, c * TCHUNK:(c + 1) * TCHUNK, :], in_=p)

        def em_t(t):
            return em_all[:, t, 0:S]

        def step(t):
            # u[s] = A[s] + A[s-1]           (state s at free index s+2)
            nc.vector.tensor_add(out=t_u, in0=t_A[:, 2:2 + S], in1=t_A[:, 1:1 + S])
            # w[s] = u[s] + skip[s]*A[s-2]   (A2 read at free index s)
            nc.vector.tensor_add(out=t_w, in0=t_u, in1=t_A2[:, 0:S])
            # A'[s] = w[s] * p_t[s]
            nc.vector.tensor_mul(out=t_A[:, 2:2 + S], in0=t_w, in1=em_t(t))
            # A2'[x] = A'[x] * skip[x]
            nc.vector.tensor_mul(out=t_A2[:, 2:2 + S], in0=t_A[:, 2:2 + S],
                                 in1=t_kk[:, 2:2 + S])

        def rescale(e):
            # s_e = sum(A) -> shist[:, e]; A *= 1/s_e; A2 *= 1/s_e
            nc.vector.tensor_reduce(out=t_shist[:, e:e + 1], in_=t_A,
                                    axis=mybir.AxisListType.X,
                                    op=mybir.AluOpType.add)
            nc.vector.reciprocal(out=t_r, in_=t_shist[:, e:e + 1])
            nc.vector.tensor_scalar_mul(out=t_A, in0=t_A, scalar1=t_r)
            nc.vector.tensor_scalar_mul(out=t_A2, in0=t_A2, scalar1=t_r)

        if variant == "noop":
            nc.vector.memset(t_A, 0.0)
            nc.vector.memset(t_cacc, 0.0)
            nc.sync.dma_start(out=out_part, in_=t_A[:, 0:1])
            nc.sync.dma_start(out=out_cacc, in_=t_cacc)
        else:
            t_aw = smalls.tile([B_LOC, AW], f32)
            t_v = smalls.tile([B_LOC, 1], f32)
            t_lnv = smalls.tile([B_LOC, 1], f32)
            t_lnh = smalls.tile([B_LOC, n_resc + 1], f32)
            t_rlog = smalls.tile([B_LOC, 1], f32)
            t_loss = smalls.tile([B_LOC, 1], f32)
            for _rep in range(repeat):
                nc.vector.memset(t_cacc, 0.0)
                nc.vector.memset(t_shist, 1.0)
                nc.vector.memset(t_A, 0.0)
                nc.vector.memset(t_A2, 0.0)
                # --- chunk 0 prep, init, then interleaved chunks/steps ---
                prep_chunk(0)
                # alpha init at t=0: A[state 0] = p0[0], A[state 1] = p0[1]
                nc.vector.tensor_copy(out=t_A[:, 2:4], in_=em_all[:, 0, 0:2])
                nc.vector.tensor_mul(out=t_A2[:, 2:4], in0=t_A[:, 2:4],
                                     in1=t_kk[:, 2:4])

                for c in range(1, NCHUNK):
                    prep_chunk(c)
                if variant not in ("noloop", "prep_only"):
                    for t in range(1, T_MAX):
                        step(t)
                        if t % RESCALE_K == 0 and t < T_MAX - 1:
                            rescale(t // RESCALE_K - 1)

                # --- final extraction ---
                nc.vector.tensor_mul(out=t_aw, in0=t_A, in1=t_wm)
                nc.vector.tensor_reduce(out=t_v, in_=t_aw,
                                        axis=mybir.AxisListType.X,
                                        op=mybir.AluOpType.add)
                nc.scalar.activation(out=t_lnv, in_=t_v,
                                     func=mybir.ActivationFunctionType.Ln)
                nc.scalar.activation(out=t_lnh, in_=t_shist,
                                     func=mybir.ActivationFunctionType.Ln)
                nc.vector.tensor_reduce(out=t_rlog, in_=t_lnh,
                                        axis=mybir.AxisListType.X,
                                        op=mybir.AluOpType.add)
                nc.vector.tensor_add(out=t_loss, in0=t_lnv, in1=t_rlog)
                nc.vector.tensor_scalar_mul(out=t_loss, in0=t_loss,
                                            scalar1=-1.0)
                nc.sync.dma_start(out=out_part, in_=t_loss)
                nc.sync.dma_start(out=out_cacc, in_=t_cacc)

    nc.compile()
    return nc


def _host_tables(tg, il, tl):
    """Per-core aux tensors from int metadata. tg [8,32], il/tl [8]."""
    f32 = np.float32
    n = B_LOC
    lmask = np.arange(L_MAX)[None, :] < tl[:, None]
    tgt = np.where(lmask, tg, 0)
    ext = np.zeros((n, S), dtype=np.int64)
    ext[:, 1::2] = tgt
    ext_shift2 = np.concatenate([np.full((n, 2), -1), ext[:, :-2]], axis=1)
    skip = (ext != 0) & (ext != ext_shift2)          # [8, 65]

    idxtab = np.zeros((128, NIDX // 16), np.int16)
    for b in range(n):
        for j in range(NIDX):
            idxtab[16 * b + j % 16, j // 16] = ext[b, j] if j < S else 0

    tt = (np.arange(NCHUNK)[None, :] * TCHUNK
          + (np.arange(128) % 16)[:, None])          # t at [(b,tau), chunk]
    bb = np.arange(128) // 16                        # sample per partition
    live = tt < il[bb][:, None]                      # [128, 16]
    bias2 = np.where(live, f32(0), f32(-1e9)).astype(f32)
    mlive = live.astype(f32)

    sstar = 2 * tl                                   # [8]
    deadhot = np.zeros((128, NCHUNK, NIDX), f32)
    sm = (np.arange(NIDX)[None, None, :] == sstar[bb][:, None, None])
    deadhot[:] = (~live)[:, :, None] & sm

    kkpad = np.zeros((n, AW), f32)
    kkpad[:, :S] = skip.astype(f32)                  # skip[state x] at free x
    wmask = np.zeros((n, AW), f32)
    wmask[np.arange(n), sstar + 2] = 1.0             # state 2tl   (idx s+2)
    wmask[np.arange(n), sstar + 1] = 1.0             # state 2tl-1
    return dict(idxtab=idxtab, bias2=bias2, mlive=mlive,
                deadhot=deadhot, kkpad=kkpad, wmask=wmask)


def kernel(log_probs, targets, input_lengths, target_lengths):
    from concourse import bass_utils

    lp = np.ascontiguousarray(np.asarray(log_probs, dtype=np.float32))
    tg = np.asarray(targets).astype(np.int64)
    il = np.asarray(input_lengths).astype(np.int64)
    tl = np.asarray(target_lengths).astype(np.int64)

    if "nc" not in _BUILD_CACHE:
        _BUILD_CACHE["nc"] = _build_program()
    nc = _BUILD_CACHE["nc"]

    in_maps = []
    for m in range(N_CORES):
        bs = slice(m * B_LOC, (m + 1) * B_LOC)
        tabs = _host_tables(tg[bs], il[bs], tl[bs])
        in_maps.append({
            "lp": np.ascontiguousarray(lp[:, bs, :]),
            "idxtab": tabs["idxtab"],
            "bias2": tabs["bias2"],
            "mlive": tabs["mlive"],
            "deadhot": np.ascontiguousarray(tabs["deadhot"]),
            "kkpad": tabs["kkpad"],
            "wmask": tabs["wmask"],
        })

    res = bass_utils.run_bass_kernel_spmd(nc, in_maps,
                                          core_ids=list(range(N_CORES)))
    losses = np.zeros(B_FULL, np.float32)
    for m in range(N_CORES):
        part = res.results[m]["loss_part"]          # [8] = -(ln v + sum ln s)
        cacc = res.results[m]["cacc"]               # [128] per-lane -sum(m_t)
        losses[m * B_LOC:(m + 1) * B_LOC] = (
            part + cacc.reshape(B_LOC, 16).sum(axis=1))
    safe = np.maximum(tl, 1).astype(np.float32)
    return np.float32(np.mean((losses / safe).astype(np.float32)))

